# revision 13
# baseline (speedup 1.0000x reference)
"""GCN (3-layer message passing) Trainium2 Bass kernel, 8-way node-sharded.

Strategy:
  - Shard nodes row-blocks across 8 cores (6250 rows each).
  - h0 = x @ W + b computed on-device (host pre-transposes x per core).
  - Per layer: AllGather the scaled feature table xs = norm*h to every
    core's HBM; dma_gather per-edge source features (256B rows) into SBUF;
    segment-sum via one-hot matmuls on the TensorEngine (K=128 edges per
    chunk), accumulating into per-128-row-block PSUM tiles; DVE applies
    (agg + xs_prev) * norm.
  - Host precomputes (from edge_index only): per-core edge chunking,
    int16 gather indices (table split in two halves to satisfy int16),
    one-hot row-relative tables, and norm = rsqrt(1+deg).

Self-contained: hardcodes the problem shapes; only needs numpy + the
concourse stack at /opt/trn_rl_repo.
"""

import sys

for _p in ("/opt/trn_rl_repo",):
    if _p not in sys.path:
        sys.path.insert(0, _p)

from dataclasses import dataclass, field

import numpy as np


@dataclass
class Cfg:
    N: int = 50000
    IN: int = 128
    HID: int = 64
    LAYERS: int = 3
    CORES: int = 8
    HALF: int = 32768  # int16 gather index limit -> table split point
    GB: int = 4        # blocks per processing group
    MAXC: int = 12     # max chunks (x128 idxs) per dma_gather call

    @property
    def NPC(self):  # nodes per core
        assert self.N % self.CORES == 0
        return self.N // self.CORES

    @property
    def BPC(self):  # 128-row blocks per core
        return (self.NPC + 127) // 128

    @property
    def NPAD(self):
        return self.BPC * 128

    @property
    def NGROUPS(self):
        return (self.BPC + self.GB - 1) // self.GB

    def group_blocks(self, g):
        return list(range(g * self.GB, min((g + 1) * self.GB, self.BPC)))


@dataclass
class Sched:
    """Per-(block, half) chunk-slot counts, shared across all cores."""
    slots: np.ndarray  # [BPC, 2] int

    # derived layout (filled by finalize): global slot ordering is
    # group-major; within a group: half 0 slots (block-major), then half 1.
    slot_block: list = field(default_factory=list)   # slot -> block
    slot_half: list = field(default_factory=list)    # slot -> half
    group_call_slots: list = field(default_factory=list)  # [g][h] -> (s0, s1)
    block_slot_ranges: list = field(default_factory=list)  # [b][h] -> (s0, s1)
    total: int = 0

    def finalize(self, cfg: Cfg):
        self.slot_block, self.slot_half = [], []
        self.group_call_slots = []
        self.block_slot_ranges = [[None, None] for _ in range(cfg.BPC)]
        s = 0
        for g in range(cfg.NGROUPS):
            calls = []
            for h in (0, 1):
                s0 = s
                for b in cfg.group_blocks(g):
                    bs0 = s
                    for _ in range(int(self.slots[b, h])):
                        self.slot_block.append(b)
                        self.slot_half.append(h)
                        s += 1
                    self.block_slot_ranges[b][h] = (bs0, s)
                calls.append((s0, s))
            self.group_call_slots.append(calls)
        self.total = s


def make_schedule(edge_index: np.ndarray, cfg: Cfg):
    """Compute the shared slot schedule + per-core slot contents."""
    row = np.asarray(edge_index[0], dtype=np.int64)
    col = np.asarray(edge_index[1], dtype=np.int64)

    core = row // cfg.NPC
    rloc = row % cfg.NPC
    blk = rloc // 128
    rrel = (rloc % 128).astype(np.float32)
    # column ids live in PADDED table space: core-major, NPAD rows per core
    cpad = (col // cfg.NPC) * cfg.NPAD + (col % cfg.NPC)
    half = (cpad >= cfg.HALF).astype(np.int64)
    cidx = (cpad - half * cfg.HALF).astype(np.int16)

    # group edges by (core, block, half); sort by source id within each
    # segment so gather descriptors walk ascending HBM addresses
    key = (core * cfg.BPC + blk) * 2 + half
    order = np.lexsort((cidx, key))
    key_s = key[order]
    rrel_s = rrel[order]
    cidx_s = cidx[order]
    half_s = half[order]

    bounds = np.searchsorted(
        key_s, np.arange(cfg.CORES * cfg.BPC * 2 + 1), side="left"
    )

    counts = np.zeros((cfg.CORES, cfg.BPC, 2), dtype=np.int64)
    for c in range(cfg.CORES):
        for b in range(cfg.BPC):
            for h in (0, 1):
                k = (c * cfg.BPC + b) * 2 + h
                counts[c, b, h] = bounds[k + 1] - bounds[k]

    slots = np.zeros((cfg.BPC, 2), dtype=np.int64)
    for b in range(cfg.BPC):
        for h in (0, 1):
            slots[b, h] = int(np.max((counts[:, b, h] + 127) // 128))
    # every block needs at least one slot so its PSUM gets initialized
    for b in range(cfg.BPC):
        if slots[b].sum() == 0:
            slots[b, 0] = 1

    sched = Sched(slots=slots)
    sched.finalize(cfg)

    # per-core slot contents
    TC = sched.total
    per_core = []
    for c in range(cfg.CORES):
        idx_flat = np.zeros((TC, 128), dtype=np.int16)
        rr_flat = np.full((TC, 128), -100.0, dtype=np.float32)
        for b in range(cfg.BPC):
            for h in (0, 1):
                k = (c * cfg.BPC + b) * 2 + h
                e0, e1 = bounds[k], bounds[k + 1]
                n = e1 - e0
                s0, s1 = sched.block_slot_ranges[b][h]
                cap = (s1 - s0) * 128
                assert n <= cap, (c, b, h, n, cap)
                ci = np.zeros(cap, dtype=np.int16)
                rv = np.full(cap, -100.0, dtype=np.float32)
                ci[:n] = cidx_s[e0:e1]
                rv[:n] = rrel_s[e0:e1]
                idx_flat[s0:s1] = ci.reshape(-1, 128)
                rr_flat[s0:s1] = rv.reshape(-1, 128)

        # gather index tile layout: [128, TC*8] int16; logical edge k of
        # slot s lives at [k % 16, s*8 + k // 16], replicated across the
        # eight 16-partition groups.
        idx_tile = np.zeros((128, TC * 8), dtype=np.int16)
        base = idx_flat.reshape(TC, 8, 16).transpose(2, 0, 1).reshape(16, TC * 8)
        for rep in range(8):
            idx_tile[rep * 16:(rep + 1) * 16] = base

        # rowrel tile: [128, TC]; partition = edge position in slot.
        rr_tile = rr_flat.T.copy()  # [128, TC]

        per_core.append((idx_tile, rr_tile))

    return sched, per_core


def host_inputs(x, edge_index, W, b, cfg: Cfg):
    """Build per-core in_maps (numpy only)."""
    x = np.asarray(x, dtype=np.float32)
    W = np.asarray(W, dtype=np.float32)
    b = np.asarray(b, dtype=np.float32)

    sched, per_core = make_schedule(edge_index, cfg)

    row = np.asarray(edge_index[0], dtype=np.int64)
    deg = np.bincount(row, minlength=cfg.N).astype(np.float32)
    norm = 1.0 / np.sqrt(1.0 + deg)

    iota = np.tile(np.arange(128, dtype=np.float32), (128, 1))
    brep = np.tile(b[None, :], (128, 1)).astype(np.float32)

    in_maps = []
    for c in range(cfg.CORES):
        r0 = c * cfg.NPC
        xT = np.zeros((cfg.IN, cfg.NPAD), dtype=np.float32)
        xT[:, : cfg.NPC] = x[r0 : r0 + cfg.NPC].T
        nc_ = np.ones(cfg.NPAD, dtype=np.float32)
        nc_[: cfg.NPC] = norm[r0 : r0 + cfg.NPC]
        normc = nc_.reshape(cfg.BPC, 128).T.copy()        # [128, BPC]
        norm2c = (normc * normc).copy()
        idx_tile, rr_tile = per_core[c]
        in_maps.append(
            {
                "xT": xT,
                "Wm": W.copy(),
                "brep": brep.copy(),
                "iota": iota.copy(),
                "normc": normc,
                "norm2c": norm2c,
                "idx_all": idx_tile,
                "rr_all": rr_tile,
            }
        )
    return sched, in_maps


def build_bass(cfg: Cfg, sched: Sched):
    """Emit the Tile program. Returns compiled nc."""
    from concourse import bacc, bass, mybir, tile

    f32 = mybir.dt.float32
    i16 = mybir.dt.int16
    EQ = mybir.AluOpType.is_equal
    ADD = mybir.AluOpType.add

    TC = sched.total
    nc = bacc.Bacc(
        "TRN2",
        target_bir_lowering=False,
        debug=False,
        num_devices=cfg.CORES,
    )

    # ---- I/O ----
    xT_d = nc.dram_tensor("xT", [cfg.IN, cfg.NPAD], f32, kind="ExternalInput")
    W_d = nc.dram_tensor("Wm", [cfg.IN, cfg.HID], f32, kind="ExternalInput")
    brep_d = nc.dram_tensor("brep", [128, cfg.HID], f32, kind="ExternalInput")
    iota_d = nc.dram_tensor("iota", [128, 128], f32, kind="ExternalInput")
    normc_d = nc.dram_tensor("normc", [128, cfg.BPC], f32, kind="ExternalInput")
    norm2c_d = nc.dram_tensor("norm2c", [128, cfg.BPC], f32, kind="ExternalInput")
    idx_d = nc.dram_tensor("idx_all", [128, TC * 8], i16, kind="ExternalInput")
    rr_d = nc.dram_tensor("rr_all", [128, TC], f32, kind="ExternalInput")
    out_d = nc.dram_tensor("out", [cfg.NPAD, cfg.HID], f32, kind="ExternalOutput")

    # internal DRAM: AG input (own xs rows) + AG output (full table), per layer
    xs_in = [
        nc.dram_tensor(f"xs_in{l}", [cfg.NPAD, cfg.HID], f32)
        for l in range(cfg.LAYERS)
    ]
    # Shared-output AG is only supported for >4-core groups
    _aspace = "Shared" if cfg.CORES > 4 else "Local"
    NTAB = cfg.CORES * cfg.NPAD  # padded table rows
    xs_full = [
        nc.dram_tensor(
            f"xs_full{l}", [NTAB, cfg.HID], f32, addr_space=_aspace
        )
        for l in range(cfg.LAYERS)
    ]

    rg = [list(range(cfg.CORES))]

    with tile.TileContext(nc) as tc:
        with (
            tc.tile_pool(name="const", bufs=1) as constp,
            tc.tile_pool(name="gbuf", bufs=2) as gpool,
            tc.tile_pool(name="onehot", bufs=2) as opool,
            tc.tile_pool(name="xsg", bufs=2) as xsgp,
            tc.tile_pool(name="psum", bufs=8, space="PSUM") as psp,
        ):
            # ---- persistent SBUF ----
            W_s = constp.tile([cfg.IN, cfg.HID], f32, tag="W")
            brep_s = constp.tile([128, cfg.HID], f32, tag="brep")
            iota_s = constp.tile([128, 128], f32, tag="iota")
            normc_s = constp.tile([128, cfg.BPC], f32, tag="normc")
            norm2c_s = constp.tile([128, cfg.BPC], f32, tag="norm2c")
            idx_s = constp.tile([128, TC * 8], i16, tag="idx")
            rr_s = constp.tile([128, TC], f32, tag="rr")
            xs_ping = constp.tile([128, cfg.BPC, cfg.HID], f32, tag="xsA")
            xs_pong = constp.tile([128, cfg.BPC, cfg.HID], f32, tag="xsB")

            nc.sync.dma_start(W_s[:], W_d[:, :])
            nc.sync.dma_start(brep_s[:], brep_d[:, :])
            nc.sync.dma_start(iota_s[:], iota_d[:, :])
            nc.sync.dma_start(normc_s[:], normc_d[:, :])
            nc.sync.dma_start(norm2c_s[:], norm2c_d[:, :])
            nc.sync.dma_start(idx_s[:], idx_d[:, :])
            nc.sync.dma_start(rr_s[:], rr_d[:, :])

            def store_group_to_dram(dram, g, src_tile, nhid):
                """src_tile [128, nb, nhid] -> dram rows [128*b0, 128*b1)."""
                blocks = cfg.group_blocks(g)
                b0, b1 = blocks[0], blocks[-1] + 1
                dst = dram[128 * b0 : 128 * b1, :].rearrange(
                    "(b p) h -> p b h", p=128
                )
                nc.sync.dma_start(dst, src_tile[:, : b1 - b0, :])

            # ---- prologue: h0 = x @ W + b ; xs0 = norm * h0 ----
            with tc.tile_pool(name="xtp", bufs=2) as xtp:
                xT_s = constp.tile([cfg.IN, cfg.NPAD], f32, tag="xT")
                nc.sync.dma_start(xT_s[:], xT_d[:, :])
                for g in range(cfg.NGROUPS):
                    blocks = cfg.group_blocks(g)
                    xs_g = xsgp.tile([128, len(blocks), cfg.HID], f32, tag="xsg")
                    for j, b in enumerate(blocks):
                        ps = psp.tile([128, cfg.HID], f32, tag="ps")
                        nc.tensor.matmul(
                            ps[:],
                            xT_s[:, 128 * b : 128 * (b + 1)],
                            W_s[:],
                            start=True,
                            stop=True,
                        )
                        tmp = xtp.tile([128, cfg.HID], f32, tag="t0")
                        nc.vector.tensor_tensor(tmp[:], ps[:], brep_s[:], ADD)
                        nc.vector.tensor_scalar_mul(
                            xs_g[:, j, :], tmp[:], normc_s[:, b : b + 1]
                        )
                        nc.vector.tensor_copy(xs_ping[:, b, :], xs_g[:, j, :])
                    store_group_to_dram(xs_in[0], g, xs_g, cfg.HID)

            nc.gpsimd.collective_compute(
                "AllGather",
                mybir.AluOpType.bypass,
                replica_groups=rg,
                ins=[xs_in[0][:, :]],
                outs=[xs_full[0][:, :]],
            )

            # ---- layers ----
            xs_cur, xs_nxt = xs_ping, xs_pong
            for l in range(cfg.LAYERS):
                last = l == cfg.LAYERS - 1
                table = xs_full[l]
                for g in range(cfg.NGROUPS):
                    blocks = cfg.group_blocks(g)
                    calls = sched.group_call_slots[g]
                    gt = {}
                    for h in (0, 1):
                        s0, s1 = calls[h]
                        nch = s1 - s0
                        if nch == 0:
                            continue
                        G = gpool.tile([128, nch, cfg.HID], f32, tag=f"G{h}")
                        src = (
                            table[: cfg.HALF, :]
                            if h == 0
                            else table[cfg.HALF :, :]
                        )
                        # split into sub-calls: the gather ucode's descriptor
                        # ring chokes on >~1.3k idxs per instruction
                        for o0 in range(0, nch, cfg.MAXC):
                            o1 = min(o0 + cfg.MAXC, nch)
                            n = o1 - o0
                            nc.gpsimd.dma_gather(
                                G[:, o0:o1, :],
                                src,
                                idx_s[:, 8 * (s0 + o0) : 8 * (s0 + o1)],
                                n * 128,
                                n * 128,
                                cfg.HID,
                                single_packet=False,
                            )
                        oh = opool.tile([128, nch, 128], f32, tag=f"oh{h}")
                        rr_b = (
                            rr_s[:, s0:s1]
                            .unsqueeze(2)
                            .broadcast_to([128, nch, 128])
                        )
                        io_b = (
                            iota_s[:, :]
                            .unsqueeze(1)
                            .broadcast_to([128, nch, 128])
                        )
                        nc.vector.tensor_tensor(oh[:], io_b, rr_b, EQ)
                        gt[h] = (G, oh, s0)

                    xs_g = xsgp.tile([128, len(blocks), cfg.HID], f32, tag="xsg")
                    for j, b in enumerate(blocks):
                        ps = psp.tile([128, cfg.HID], f32, tag="ps")
                        # all slots of this block, halves concatenated
                        mm = []
                        for h in (0, 1):
                            bs0, bs1 = sched.block_slot_ranges[b][h]
                            if bs1 > bs0:
                                G, oh, s0 = gt[h]
                                for s in range(bs0, bs1):
                                    mm.append((G, oh, s - s0))
                        assert mm, f"block {b} has no slots"
                        for k, (G, oh, sl) in enumerate(mm):
                            nc.tensor.matmul(
                                ps[:],
                                oh[:, sl, :],
                                G[:, sl, :],
                                start=(k == 0),
                                stop=(k == len(mm) - 1),
                            )
                        # out = (agg + xs_cur[b]) * (norm2 or norm)
                        nsrc = normc_s if last else norm2c_s
                        tmp = xsgp.tile([128, cfg.HID], f32, tag="t1")
                        nc.vector.tensor_tensor(
                            tmp[:], ps[:], xs_cur[:, b, :], ADD
                        )
                        nc.vector.tensor_scalar_mul(
                            xs_g[:, j, :], tmp[:], nsrc[:, b : b + 1]
                        )
                        if not last:
                            nc.vector.tensor_copy(xs_nxt[:, b, :], xs_g[:, j, :])
                    if last:
                        store_group_to_dram(out_d, g, xs_g, cfg.HID)
                    else:
                        store_group_to_dram(xs_in[l + 1], g, xs_g, cfg.HID)

                if not last:
                    nc.gpsimd.collective_compute(
                        "AllGather",
                        mybir.AluOpType.bypass,
                        replica_groups=rg,
                        ins=[xs_in[l + 1][:, :]],
                        outs=[xs_full[l + 1][:, :]],
                    )
                    xs_cur, xs_nxt = xs_nxt, xs_cur

    nc.compile()
    return nc


def bench_bass(nc, in_maps, n_cores, iters=20, warmup=2):
    """Repeat-execution device benchmark (no NTFF tracing in this container).

    Mirrors bass2jax.run_bass_via_pjrt's multi-core path, minus output-buffer
    donation so the compiled executable can be re-invoked. Returns
    (results_list, per_iter_seconds).
    """
    import time

    import jax
    from jax.experimental.shard_map import shard_map
    from jax.sharding import Mesh, NamedSharding, PartitionSpec

    from concourse import bass2jax, mybir

    bass2jax.install_neuronx_cc_hook()

    partition_name = (
        nc.partition_id_tensor.name if nc.partition_id_tensor else None
    )
    in_names, out_names, out_avals, zero_outs = [], [], [], []
    for alloc in nc.m.functions[0].allocations:
        if not isinstance(alloc, mybir.MemoryLocationSet):
            continue
        name = alloc.memorylocations[0].name
        if alloc.kind == "ExternalInput":
            if name != partition_name:
                in_names.append(name)
        elif alloc.kind == "ExternalOutput":
            out_names.append(name)
            shape = tuple(alloc.tensor_shape)
            dtype = mybir.dt.np(alloc.dtype)
            out_avals.append(jax.core.ShapedArray(shape, dtype))
            zero_outs.append(np.zeros(shape, dtype))
    n_params = len(in_names)
    all_names = in_names + out_names
    if partition_name is not None:
        all_names = all_names + [partition_name]

    def _body(*args):
        operands = list(args)
        if partition_name is not None:
            operands.append(bass2jax.partition_id_tensor())
        outs = bass2jax._bass_exec_p.bind(
            *operands,
            out_avals=tuple(out_avals),
            in_names=tuple(all_names),
            out_names=tuple(out_names),
            lowering_input_output_aliases=(),
            sim_require_finite=True,
            sim_require_nnan=True,
            nc=nc,
        )
        return tuple(outs)

    devices = jax.devices()[:n_cores]
    mesh = Mesh(np.asarray(devices), ("core",))
    spec = PartitionSpec("core")
    nin = n_params + len(zero_outs)
    sharded = jax.jit(
        shard_map(
            _body,
            mesh=mesh,
            in_specs=(spec,) * nin,
            out_specs=(spec,) * len(out_names),
            check_rep=False,
        ),
        keep_unused=True,
    )
    sh = NamedSharding(mesh, spec)
    args = [
        jax.device_put(
            np.concatenate([np.asarray(m[name]) for m in in_maps], axis=0), sh
        )
        for name in in_names
    ] + [
        jax.device_put(
            np.zeros((n_cores * z.shape[0], *z.shape[1:]), z.dtype), sh
        )
        for z in zero_outs
    ]

    out_arrs = None
    for _ in range(warmup):
        out_arrs = sharded(*args)
        jax.block_until_ready(out_arrs)
    t0 = time.perf_counter()
    for _ in range(iters):
        out_arrs = sharded(*args)
    jax.block_until_ready(out_arrs)
    t1 = time.perf_counter()

    results = [
        {
            name: np.asarray(out_arrs[i]).reshape(n_cores, *out_avals[i].shape)[c]
            for i, name in enumerate(out_names)
        }
        for c in range(n_cores)
    ]
    return results, (t1 - t0) / iters


def kernel(x, edge_index, W, b, cfg: Cfg | None = None, trace: bool = False):
    """Full-input entry point. Returns [N, HID] float32 (+ results if trace)."""
    cfg = cfg or Cfg()
    sched, in_maps = host_inputs(x, edge_index, W, b, cfg)
    nc = build_bass(cfg, sched)

    from concourse import bass_utils

    res = bass_utils.run_bass_kernel_spmd(
        nc,
        in_maps,
        core_ids=list(range(cfg.CORES)),
        trace=False,
    )
    out = np.concatenate(
        [r["out"][: cfg.NPC] for r in res.results], axis=0
    )
    if trace:
        return out, res
    return out


if __name__ == "__main__":
    pass


# revision 19
# speedup vs baseline: 3.3147x; 3.3147x over previous
"""GCN (3-layer message passing) Trainium2 Bass kernel, 8-way node-sharded.

Strategy:
  - Shard nodes row-blocks across 8 cores (6250 rows each).
  - h0 = x @ W + b computed on-device (host pre-transposes x per core).
  - Per layer: AllGather the scaled feature table xs = norm*h to every
    core's HBM; dma_gather per-edge source features (256B rows) into SBUF;
    segment-sum via one-hot matmuls on the TensorEngine (K=128 edges per
    chunk), accumulating into per-128-row-block PSUM tiles; DVE applies
    (agg + xs_prev) * norm.
  - Host precomputes (from edge_index only): per-core edge chunking,
    int16 gather indices (table split in two halves to satisfy int16),
    one-hot row-relative tables, and norm = rsqrt(1+deg).

Self-contained: hardcodes the problem shapes; only needs numpy + the
concourse stack at /opt/trn_rl_repo.
"""

import sys

for _p in ("/opt/trn_rl_repo",):
    if _p not in sys.path:
        sys.path.insert(0, _p)

from dataclasses import dataclass, field

import numpy as np


@dataclass
class Cfg:
    N: int = 50000
    IN: int = 128
    HID: int = 64
    LAYERS: int = 3
    CORES: int = 8
    HALF: int = 32768  # int16 gather index limit -> table split point
    GB: int = 4        # blocks per processing group
    MAXC: int = 12     # max chunks (x128 idxs) per dma_gather call
    REPEAT: int = 1    # repeat the layer stack (timing experiments only)

    @property
    def NPC(self):  # nodes per core
        assert self.N % self.CORES == 0
        return self.N // self.CORES

    @property
    def BPC(self):  # 128-row blocks per core
        return (self.NPC + 127) // 128

    @property
    def NPAD(self):
        return self.BPC * 128

    @property
    def NGROUPS(self):
        return (self.BPC + self.GB - 1) // self.GB

    def group_blocks(self, g):
        return list(range(g * self.GB, min((g + 1) * self.GB, self.BPC)))


@dataclass
class Sched:
    """Per-(block, half) chunk-slot counts, shared across all cores."""
    slots: np.ndarray  # [BPC, 2] int

    # derived layout (filled by finalize): global slot ordering is
    # group-major; within a group: half 0 slots (block-major), then half 1.
    slot_block: list = field(default_factory=list)   # slot -> block
    slot_half: list = field(default_factory=list)    # slot -> half
    group_call_slots: list = field(default_factory=list)  # [g][h] -> (s0, s1)
    block_slot_ranges: list = field(default_factory=list)  # [b][h] -> (s0, s1)
    total: int = 0

    def finalize(self, cfg: Cfg):
        self.slot_block, self.slot_half = [], []
        self.group_call_slots = []
        self.block_slot_ranges = [[None, None] for _ in range(cfg.BPC)]
        s = 0
        for g in range(cfg.NGROUPS):
            calls = []
            for h in (0, 1):
                s0 = s
                for b in cfg.group_blocks(g):
                    bs0 = s
                    for _ in range(int(self.slots[b, h])):
                        self.slot_block.append(b)
                        self.slot_half.append(h)
                        s += 1
                    self.block_slot_ranges[b][h] = (bs0, s)
                calls.append((s0, s))
            self.group_call_slots.append(calls)
        self.total = s


def make_schedule(edge_index: np.ndarray, cfg: Cfg):
    """Compute the shared slot schedule + per-core slot contents."""
    row = np.asarray(edge_index[0], dtype=np.int64)
    col = np.asarray(edge_index[1], dtype=np.int64)

    core = row // cfg.NPC
    rloc = row % cfg.NPC
    blk = rloc // 128
    rrel = (rloc % 128).astype(np.float32)
    # column ids live in PADDED table space: core-major, NPAD rows per core
    cpad = (col // cfg.NPC) * cfg.NPAD + (col % cfg.NPC)
    half = (cpad >= cfg.HALF).astype(np.int64)
    cidx = (cpad - half * cfg.HALF).astype(np.int16)

    # group edges by (core, block, half); sort by source id within each
    # segment so gather descriptors walk ascending HBM addresses
    key = (core * cfg.BPC + blk) * 2 + half
    order = np.lexsort((cidx, key))
    key_s = key[order]
    rrel_s = rrel[order]
    cidx_s = cidx[order]
    half_s = half[order]

    bounds = np.searchsorted(
        key_s, np.arange(cfg.CORES * cfg.BPC * 2 + 1), side="left"
    )

    counts = np.zeros((cfg.CORES, cfg.BPC, 2), dtype=np.int64)
    for c in range(cfg.CORES):
        for b in range(cfg.BPC):
            for h in (0, 1):
                k = (c * cfg.BPC + b) * 2 + h
                counts[c, b, h] = bounds[k + 1] - bounds[k]

    slots = np.zeros((cfg.BPC, 2), dtype=np.int64)
    for b in range(cfg.BPC):
        for h in (0, 1):
            slots[b, h] = int(np.max((counts[:, b, h] + 127) // 128))
    # every block needs at least one slot so its PSUM gets initialized
    for b in range(cfg.BPC):
        if slots[b].sum() == 0:
            slots[b, 0] = 1

    sched = Sched(slots=slots)
    sched.finalize(cfg)

    # per-core slot contents
    TC = sched.total
    per_core = []
    for c in range(cfg.CORES):
        idx_flat = np.zeros((TC, 128), dtype=np.int16)
        rr_flat = np.full((TC, 128), -100.0, dtype=np.float32)
        for b in range(cfg.BPC):
            for h in (0, 1):
                k = (c * cfg.BPC + b) * 2 + h
                e0, e1 = bounds[k], bounds[k + 1]
                n = e1 - e0
                s0, s1 = sched.block_slot_ranges[b][h]
                cap = (s1 - s0) * 128
                assert n <= cap, (c, b, h, n, cap)
                ci = np.zeros(cap, dtype=np.int16)
                rv = np.full(cap, -100.0, dtype=np.float32)
                ci[:n] = cidx_s[e0:e1]
                rv[:n] = rrel_s[e0:e1]
                idx_flat[s0:s1] = ci.reshape(-1, 128)
                rr_flat[s0:s1] = rv.reshape(-1, 128)

        # gather index tile layout: [128, TC*8] int16; logical edge k of
        # slot s lives at [k % 16, s*8 + k // 16], replicated across the
        # eight 16-partition groups.
        idx_tile = np.zeros((128, TC * 8), dtype=np.int16)
        base = idx_flat.reshape(TC, 8, 16).transpose(2, 0, 1).reshape(16, TC * 8)
        for rep in range(8):
            idx_tile[rep * 16:(rep + 1) * 16] = base

        # rowrel tile: [128, TC]; partition = edge position in slot.
        rr_tile = rr_flat.T.copy()  # [128, TC]

        per_core.append((idx_tile, rr_tile))

    return sched, per_core


def host_inputs(x, edge_index, W, b, cfg: Cfg):
    """Build per-core in_maps (numpy only)."""
    x = np.asarray(x, dtype=np.float32)
    W = np.asarray(W, dtype=np.float32)
    b = np.asarray(b, dtype=np.float32)

    sched, per_core = make_schedule(edge_index, cfg)

    row = np.asarray(edge_index[0], dtype=np.int64)
    deg = np.bincount(row, minlength=cfg.N).astype(np.float32)
    norm = 1.0 / np.sqrt(1.0 + deg)

    iota = np.tile(np.arange(128, dtype=np.float32), (128, 1))
    brep = np.tile(b[None, :], (128, 1)).astype(np.float32)

    in_maps = []
    for c in range(cfg.CORES):
        r0 = c * cfg.NPC
        xT = np.zeros((cfg.IN, cfg.NPAD), dtype=np.float32)
        xT[:, : cfg.NPC] = x[r0 : r0 + cfg.NPC].T
        nc_ = np.ones(cfg.NPAD, dtype=np.float32)
        nc_[: cfg.NPC] = norm[r0 : r0 + cfg.NPC]
        normc = nc_.reshape(cfg.BPC, 128).T.copy()        # [128, BPC]
        norm2c = (normc * normc).copy()
        idx_tile, rr_tile = per_core[c]
        in_maps.append(
            {
                "xT": xT,
                "Wm": W.copy(),
                "brep": brep.copy(),
                "iota": iota.copy(),
                "normc": normc,
                "norm2c": norm2c,
                "idx_all": idx_tile,
                "rr_all": rr_tile,
            }
        )
    return sched, in_maps


def build_bass(cfg: Cfg, sched: Sched, no_ag: bool = False, ablate=()):
    """Emit the Tile program. Returns compiled nc."""
    from concourse import bacc, bass, mybir, tile

    f32 = mybir.dt.float32
    i16 = mybir.dt.int16
    EQ = mybir.AluOpType.is_equal
    ADD = mybir.AluOpType.add

    TC = sched.total
    nc = bacc.Bacc(
        "TRN2",
        target_bir_lowering=False,
        debug=False,
        num_devices=cfg.CORES,
        num_swdge_queues=4,
    )

    # ---- I/O ----
    xT_d = nc.dram_tensor("xT", [cfg.IN, cfg.NPAD], f32, kind="ExternalInput")
    W_d = nc.dram_tensor("Wm", [cfg.IN, cfg.HID], f32, kind="ExternalInput")
    brep_d = nc.dram_tensor("brep", [128, cfg.HID], f32, kind="ExternalInput")
    iota_d = nc.dram_tensor("iota", [128, 128], f32, kind="ExternalInput")
    normc_d = nc.dram_tensor("normc", [128, cfg.BPC], f32, kind="ExternalInput")
    norm2c_d = nc.dram_tensor("norm2c", [128, cfg.BPC], f32, kind="ExternalInput")
    idx_d = nc.dram_tensor("idx_all", [128, TC * 8], i16, kind="ExternalInput")
    rr_d = nc.dram_tensor("rr_all", [128, TC], f32, kind="ExternalInput")
    out_d = nc.dram_tensor("out", [cfg.NPAD, cfg.HID], f32, kind="ExternalOutput")

    # internal DRAM: AG input (own xs rows) + AG output (full table), per layer
    xs_in = [
        nc.dram_tensor(f"xs_in{l}", [cfg.NPAD, cfg.HID], f32)
        for l in range(cfg.LAYERS)
    ]
    # Shared-output AG is only supported for >4-core groups
    _aspace = "Shared" if cfg.CORES > 4 else "Local"
    NTAB = cfg.CORES * cfg.NPAD  # padded table rows
    xs_full = [
        nc.dram_tensor(
            f"xs_full{l}", [NTAB, cfg.HID], f32, addr_space=_aspace
        )
        for l in range(cfg.LAYERS)
    ]

    rg = [list(range(cfg.CORES))]

    with tile.TileContext(nc) as tc:
        with (
            tc.tile_pool(name="const", bufs=1) as constp,
            tc.tile_pool(name="gbuf", bufs=2) as gpool,
            tc.tile_pool(name="onehot", bufs=2) as opool,
            tc.tile_pool(name="xsg", bufs=2) as xsgp,
            tc.tile_pool(name="psum", bufs=8, space="PSUM") as psp,
        ):
            # ---- persistent SBUF ----
            W_s = constp.tile([cfg.IN, cfg.HID], f32, tag="W")
            brep_s = constp.tile([128, cfg.HID], f32, tag="brep")
            iota_s = constp.tile([128, 128], f32, tag="iota")
            normc_s = constp.tile([128, cfg.BPC], f32, tag="normc")
            norm2c_s = constp.tile([128, cfg.BPC], f32, tag="norm2c")
            idx_s = constp.tile([128, TC * 8], i16, tag="idx")
            rr_s = constp.tile([128, TC], f32, tag="rr")
            xs_ping = constp.tile([128, cfg.BPC, cfg.HID], f32, tag="xsA")
            xs_pong = constp.tile([128, cfg.BPC, cfg.HID], f32, tag="xsB")

            nc.sync.dma_start(W_s[:], W_d[:, :])
            nc.sync.dma_start(brep_s[:], brep_d[:, :])
            nc.sync.dma_start(iota_s[:], iota_d[:, :])
            nc.sync.dma_start(normc_s[:], normc_d[:, :])
            nc.sync.dma_start(norm2c_s[:], norm2c_d[:, :])
            nc.sync.dma_start(idx_s[:], idx_d[:, :])
            nc.sync.dma_start(rr_s[:], rr_d[:, :])

            def store_group_to_dram(dram, g, src_tile, nhid):
                """src_tile [128, nb, nhid] -> dram rows [128*b0, 128*b1)."""
                blocks = cfg.group_blocks(g)
                b0, b1 = blocks[0], blocks[-1] + 1
                dst = dram[128 * b0 : 128 * b1, :].rearrange(
                    "(b p) h -> p b h", p=128
                )
                nc.sync.dma_start(dst, src_tile[:, : b1 - b0, :])

            # ---- prologue: h0 = x @ W + b ; xs0 = norm * h0 ----
            with tc.tile_pool(name="xtp", bufs=2) as xtp:
                xT_s = constp.tile([cfg.IN, cfg.NPAD], f32, tag="xT")
                nc.sync.dma_start(xT_s[:], xT_d[:, :])
                for g in range(cfg.NGROUPS):
                    blocks = cfg.group_blocks(g)
                    xs_g = xsgp.tile([128, len(blocks), cfg.HID], f32, tag="xsg")
                    for j, b in enumerate(blocks):
                        ps = psp.tile([128, cfg.HID], f32, tag="ps")
                        nc.tensor.matmul(
                            ps[:],
                            xT_s[:, 128 * b : 128 * (b + 1)],
                            W_s[:],
                            start=True,
                            stop=True,
                        )
                        tmp = xtp.tile([128, cfg.HID], f32, tag="t0")
                        nc.vector.tensor_tensor(tmp[:], ps[:], brep_s[:], ADD)
                        nc.vector.tensor_scalar_mul(
                            xs_g[:, j, :], tmp[:], normc_s[:, b : b + 1]
                        )
                        nc.vector.tensor_copy(xs_ping[:, b, :], xs_g[:, j, :])
                    store_group_to_dram(xs_in[0], g, xs_g, cfg.HID)

            if not no_ag:
                nc.gpsimd.collective_compute(
                    "AllGather",
                    mybir.AluOpType.bypass,
                    replica_groups=rg,
                    ins=[xs_in[0][:, :]],
                    outs=[xs_full[0][:, :]],
                )

            # ---- layers ----
            xs_cur, xs_nxt = xs_ping, xs_pong
            for _rep in range(cfg.REPEAT):
              for l in range(cfg.LAYERS):
                last = l == cfg.LAYERS - 1
                table = xs_full[l]
                for g in range(cfg.NGROUPS):
                    blocks = cfg.group_blocks(g)
                    calls = sched.group_call_slots[g]
                    gt = {}
                    for h in (0, 1):
                        s0, s1 = calls[h]
                        nch = s1 - s0
                        if nch == 0:
                            continue
                        G = gpool.tile([128, nch, cfg.HID], f32, tag=f"G{h}")
                        src = (
                            table[: cfg.HALF, :]
                            if h == 0
                            else table[cfg.HALF :, :]
                        )
                        # split into sub-calls: the gather ucode's descriptor
                        # ring chokes on >~1.3k idxs per instruction
                        for ci, o0 in enumerate(
                            [] if "gather" in ablate else range(0, nch, cfg.MAXC)
                        ):
                            o1 = min(o0 + cfg.MAXC, nch)
                            n = o1 - o0
                            nc.gpsimd.dma_gather(
                                G[:, o0:o1, :],
                                src,
                                idx_s[:, 8 * (s0 + o0) : 8 * (s0 + o1)],
                                n * 128,
                                n * 128,
                                cfg.HID,
                                single_packet=False,
                                queue_num=(ci + 2 * h) % 4,
                            )
                        oh = opool.tile([128, nch, 128], f32, tag=f"oh{h}")
                        rr_b = (
                            rr_s[:, s0:s1]
                            .unsqueeze(2)
                            .broadcast_to([128, nch, 128])
                        )
                        io_b = (
                            iota_s[:, :]
                            .unsqueeze(1)
                            .broadcast_to([128, nch, 128])
                        )
                        if "oh" not in ablate:
                            nc.vector.tensor_tensor(oh[:], io_b, rr_b, EQ)
                        gt[h] = (G, oh, s0)

                    xs_g = xsgp.tile([128, len(blocks), cfg.HID], f32, tag="xsg")
                    for j, b in enumerate(blocks):
                        ps = psp.tile([128, cfg.HID], f32, tag="ps")
                        # all slots of this block, halves concatenated
                        mm = []
                        for h in (0, 1):
                            bs0, bs1 = sched.block_slot_ranges[b][h]
                            if bs1 > bs0:
                                G, oh, s0 = gt[h]
                                for s in range(bs0, bs1):
                                    mm.append((G, oh, s - s0))
                        assert mm, f"block {b} has no slots"
                        if "mm" in ablate:
                            nc.vector.memset(ps[:], 0.0)
                            mm = []
                        for k, (G, oh, sl) in enumerate(mm):
                            nc.tensor.matmul(
                                ps[:],
                                oh[:, sl, :],
                                G[:, sl, :],
                                start=(k == 0),
                                stop=(k == len(mm) - 1),
                            )
                        # out = (agg + xs_cur[b]) * (norm2 or norm)
                        nsrc = normc_s if last else norm2c_s
                        tmp = xsgp.tile([128, cfg.HID], f32, tag="t1")
                        nc.vector.tensor_tensor(
                            tmp[:], ps[:], xs_cur[:, b, :], ADD
                        )
                        nc.vector.tensor_scalar_mul(
                            xs_g[:, j, :], tmp[:], nsrc[:, b : b + 1]
                        )
                        if not last:
                            nc.vector.tensor_copy(xs_nxt[:, b, :], xs_g[:, j, :])
                    if last:
                        store_group_to_dram(out_d, g, xs_g, cfg.HID)
                    else:
                        store_group_to_dram(xs_in[l + 1], g, xs_g, cfg.HID)

                if not last:
                    if not no_ag:
                        nc.gpsimd.collective_compute(
                            "AllGather",
                            mybir.AluOpType.bypass,
                            replica_groups=rg,
                            ins=[xs_in[l + 1][:, :]],
                            outs=[xs_full[l + 1][:, :]],
                        )
                    xs_cur, xs_nxt = xs_nxt, xs_cur

    nc.compile()
    return nc


def bench_bass(nc, in_maps, n_cores, iters=20, warmup=2):
    """Repeat-execution device benchmark (no NTFF tracing in this container).

    Mirrors bass2jax.run_bass_via_pjrt's multi-core path, minus output-buffer
    donation so the compiled executable can be re-invoked. Returns
    (results_list, per_iter_seconds).
    """
    import time

    import jax
    from jax.experimental.shard_map import shard_map
    from jax.sharding import Mesh, NamedSharding, PartitionSpec

    from concourse import bass2jax, mybir

    bass2jax.install_neuronx_cc_hook()

    partition_name = (
        nc.partition_id_tensor.name if nc.partition_id_tensor else None
    )
    in_names, out_names, out_avals, zero_outs = [], [], [], []
    for alloc in nc.m.functions[0].allocations:
        if not isinstance(alloc, mybir.MemoryLocationSet):
            continue
        name = alloc.memorylocations[0].name
        if alloc.kind == "ExternalInput":
            if name != partition_name:
                in_names.append(name)
        elif alloc.kind == "ExternalOutput":
            out_names.append(name)
            shape = tuple(alloc.tensor_shape)
            dtype = mybir.dt.np(alloc.dtype)
            out_avals.append(jax.core.ShapedArray(shape, dtype))
            zero_outs.append(np.zeros(shape, dtype))
    n_params = len(in_names)
    all_names = in_names + out_names
    if partition_name is not None:
        all_names = all_names + [partition_name]

    def _body(*args):
        operands = list(args)
        if partition_name is not None:
            operands.append(bass2jax.partition_id_tensor())
        outs = bass2jax._bass_exec_p.bind(
            *operands,
            out_avals=tuple(out_avals),
            in_names=tuple(all_names),
            out_names=tuple(out_names),
            lowering_input_output_aliases=(),
            sim_require_finite=True,
            sim_require_nnan=True,
            nc=nc,
        )
        return tuple(outs)

    devices = jax.devices()[:n_cores]
    mesh = Mesh(np.asarray(devices), ("core",))
    spec = PartitionSpec("core")
    nin = n_params + len(zero_outs)
    sharded = jax.jit(
        shard_map(
            _body,
            mesh=mesh,
            in_specs=(spec,) * nin,
            out_specs=(spec,) * len(out_names),
            check_rep=False,
        ),
        keep_unused=True,
    )
    sh = NamedSharding(mesh, spec)
    args = [
        jax.device_put(
            np.concatenate([np.asarray(m[name]) for m in in_maps], axis=0), sh
        )
        for name in in_names
    ] + [
        jax.device_put(
            np.zeros((n_cores * z.shape[0], *z.shape[1:]), z.dtype), sh
        )
        for z in zero_outs
    ]

    out_arrs = None
    for _ in range(warmup):
        out_arrs = sharded(*args)
        jax.block_until_ready(out_arrs)
    t0 = time.perf_counter()
    for _ in range(iters):
        out_arrs = sharded(*args)
    jax.block_until_ready(out_arrs)
    t1 = time.perf_counter()

    results = [
        {
            name: np.asarray(out_arrs[i]).reshape(n_cores, *out_avals[i].shape)[c]
            for i, name in enumerate(out_names)
        }
        for c in range(n_cores)
    ]
    return results, (t1 - t0) / iters


def kernel(x, edge_index, W, b, cfg: Cfg | None = None, trace: bool = False):
    """Full-input entry point. Returns [N, HID] float32 (+ results if trace)."""
    cfg = cfg or Cfg()
    sched, in_maps = host_inputs(x, edge_index, W, b, cfg)
    nc = build_bass(cfg, sched)

    from concourse import bass_utils

    res = bass_utils.run_bass_kernel_spmd(
        nc,
        in_maps,
        core_ids=list(range(cfg.CORES)),
        trace=False,
    )
    out = np.concatenate(
        [r["out"][: cfg.NPC] for r in res.results], axis=0
    )
    if trace:
        return out, res
    return out


if __name__ == "__main__":
    pass


# revision 22
# speedup vs baseline: 3.7402x; 1.1283x over previous
"""GCN (3-layer message passing) Trainium2 Bass kernel, 8-way node-sharded.

Strategy:
  - Shard nodes row-blocks across 8 cores (6250 rows each).
  - h0 = x @ W + b computed on-device (host pre-transposes x per core).
  - Per layer: AllGather the scaled feature table xs = norm*h to every
    core's HBM; dma_gather per-edge source features (256B rows) into SBUF;
    segment-sum via one-hot matmuls on the TensorEngine (K=128 edges per
    chunk), accumulating into per-128-row-block PSUM tiles; DVE applies
    (agg + xs_prev) * norm.
  - Host precomputes (from edge_index only): per-core edge chunking,
    int16 gather indices (table split in two halves to satisfy int16),
    one-hot row-relative tables, and norm = rsqrt(1+deg).

Self-contained: hardcodes the problem shapes; only needs numpy + the
concourse stack at /opt/trn_rl_repo.
"""

import sys

for _p in ("/opt/trn_rl_repo",):
    if _p not in sys.path:
        sys.path.insert(0, _p)

from dataclasses import dataclass, field

import numpy as np


@dataclass
class Cfg:
    N: int = 50000
    IN: int = 128
    HID: int = 64
    LAYERS: int = 3
    CORES: int = 8
    HALF: int = 32768  # int16 gather index limit -> table split point
    GB: int = 4        # blocks per processing group
    MAXC: int = 12     # max chunks (x128 idxs) per dma_gather call
    OHCHUNK: bool = False  # build one-hots per gather sub-call vs per half
    GBUFS: int = 2     # gather-tile double/triple buffering
    REPEAT: int = 1    # repeat the layer stack (timing experiments only)

    @property
    def NPC(self):  # nodes per core
        assert self.N % self.CORES == 0
        return self.N // self.CORES

    @property
    def BPC(self):  # 128-row blocks per core
        return (self.NPC + 127) // 128

    @property
    def NPAD(self):
        return self.BPC * 128

    @property
    def NGROUPS(self):
        return (self.BPC + self.GB - 1) // self.GB

    def group_blocks(self, g):
        return list(range(g * self.GB, min((g + 1) * self.GB, self.BPC)))


@dataclass
class Sched:
    """Per-(block, half) chunk-slot counts, shared across all cores."""
    slots: np.ndarray  # [BPC, 2] int

    # derived layout (filled by finalize): global slot ordering is
    # group-major; within a group: half 0 slots (block-major), then half 1.
    slot_block: list = field(default_factory=list)   # slot -> block
    slot_half: list = field(default_factory=list)    # slot -> half
    group_call_slots: list = field(default_factory=list)  # [g][h] -> (s0, s1)
    block_slot_ranges: list = field(default_factory=list)  # [b][h] -> (s0, s1)
    total: int = 0

    def finalize(self, cfg: Cfg):
        self.slot_block, self.slot_half = [], []
        self.group_call_slots = []
        self.block_slot_ranges = [[None, None] for _ in range(cfg.BPC)]
        s = 0
        for g in range(cfg.NGROUPS):
            calls = []
            for h in (0, 1):
                s0 = s
                for b in cfg.group_blocks(g):
                    bs0 = s
                    for _ in range(int(self.slots[b, h])):
                        self.slot_block.append(b)
                        self.slot_half.append(h)
                        s += 1
                    self.block_slot_ranges[b][h] = (bs0, s)
                calls.append((s0, s))
            self.group_call_slots.append(calls)
        self.total = s


def make_schedule(edge_index: np.ndarray, cfg: Cfg):
    """Compute the shared slot schedule + per-core slot contents."""
    row = np.asarray(edge_index[0], dtype=np.int64)
    col = np.asarray(edge_index[1], dtype=np.int64)

    core = row // cfg.NPC
    rloc = row % cfg.NPC
    blk = rloc // 128
    rrel = (rloc % 128).astype(np.float32)
    # column ids live in PADDED table space: core-major, NPAD rows per core
    cpad = (col // cfg.NPC) * cfg.NPAD + (col % cfg.NPC)
    half = (cpad >= cfg.HALF).astype(np.int64)
    cidx = (cpad - half * cfg.HALF).astype(np.int16)

    # group edges by (core, block, half); sort by source id within each
    # segment so gather descriptors walk ascending HBM addresses
    key = (core * cfg.BPC + blk) * 2 + half
    order = np.lexsort((cidx, key))
    key_s = key[order]
    rrel_s = rrel[order]
    cidx_s = cidx[order]
    half_s = half[order]

    bounds = np.searchsorted(
        key_s, np.arange(cfg.CORES * cfg.BPC * 2 + 1), side="left"
    )

    counts = np.zeros((cfg.CORES, cfg.BPC, 2), dtype=np.int64)
    for c in range(cfg.CORES):
        for b in range(cfg.BPC):
            for h in (0, 1):
                k = (c * cfg.BPC + b) * 2 + h
                counts[c, b, h] = bounds[k + 1] - bounds[k]

    slots = np.zeros((cfg.BPC, 2), dtype=np.int64)
    for b in range(cfg.BPC):
        for h in (0, 1):
            slots[b, h] = int(np.max((counts[:, b, h] + 127) // 128))
    # every block needs at least one slot so its PSUM gets initialized
    for b in range(cfg.BPC):
        if slots[b].sum() == 0:
            slots[b, 0] = 1

    sched = Sched(slots=slots)
    sched.finalize(cfg)

    # per-core slot contents
    TC = sched.total
    per_core = []
    for c in range(cfg.CORES):
        idx_flat = np.zeros((TC, 128), dtype=np.int16)
        rr_flat = np.full((TC, 128), -100.0, dtype=np.float32)
        for b in range(cfg.BPC):
            for h in (0, 1):
                k = (c * cfg.BPC + b) * 2 + h
                e0, e1 = bounds[k], bounds[k + 1]
                n = e1 - e0
                s0, s1 = sched.block_slot_ranges[b][h]
                cap = (s1 - s0) * 128
                assert n <= cap, (c, b, h, n, cap)
                ci = np.zeros(cap, dtype=np.int16)
                rv = np.full(cap, -100.0, dtype=np.float32)
                ci[:n] = cidx_s[e0:e1]
                rv[:n] = rrel_s[e0:e1]
                idx_flat[s0:s1] = ci.reshape(-1, 128)
                rr_flat[s0:s1] = rv.reshape(-1, 128)

        # gather index tile layout: [128, TC*8] int16; logical edge k of
        # slot s lives at [k % 16, s*8 + k // 16], replicated across the
        # eight 16-partition groups.
        idx_tile = np.zeros((128, TC * 8), dtype=np.int16)
        base = idx_flat.reshape(TC, 8, 16).transpose(2, 0, 1).reshape(16, TC * 8)
        for rep in range(8):
            idx_tile[rep * 16:(rep + 1) * 16] = base

        # rowrel tile: [128, TC]; partition = edge position in slot.
        rr_tile = rr_flat.T.copy()  # [128, TC]

        per_core.append((idx_tile, rr_tile))

    return sched, per_core


def host_inputs(x, edge_index, W, b, cfg: Cfg):
    """Build per-core in_maps (numpy only)."""
    x = np.asarray(x, dtype=np.float32)
    W = np.asarray(W, dtype=np.float32)
    b = np.asarray(b, dtype=np.float32)

    sched, per_core = make_schedule(edge_index, cfg)

    row = np.asarray(edge_index[0], dtype=np.int64)
    deg = np.bincount(row, minlength=cfg.N).astype(np.float32)
    norm = 1.0 / np.sqrt(1.0 + deg)

    iota = np.tile(np.arange(128, dtype=np.float32), (128, 1))
    brep = np.tile(b[None, :], (128, 1)).astype(np.float32)

    in_maps = []
    for c in range(cfg.CORES):
        r0 = c * cfg.NPC
        xT = np.zeros((cfg.IN, cfg.NPAD), dtype=np.float32)
        xT[:, : cfg.NPC] = x[r0 : r0 + cfg.NPC].T
        nc_ = np.ones(cfg.NPAD, dtype=np.float32)
        nc_[: cfg.NPC] = norm[r0 : r0 + cfg.NPC]
        normc = nc_.reshape(cfg.BPC, 128).T.copy()        # [128, BPC]
        norm2c = (normc * normc).copy()
        idx_tile, rr_tile = per_core[c]
        in_maps.append(
            {
                "xT": xT,
                "Wm": W.copy(),
                "brep": brep.copy(),
                "iota": iota.copy(),
                "normc": normc,
                "norm2c": norm2c,
                "idx_all": idx_tile,
                "rr_all": rr_tile,
            }
        )
    return sched, in_maps


def build_bass(cfg: Cfg, sched: Sched, no_ag: bool = False, ablate=()):
    """Emit the Tile program. Returns compiled nc."""
    from concourse import bacc, bass, mybir, tile

    f32 = mybir.dt.float32
    i16 = mybir.dt.int16
    EQ = mybir.AluOpType.is_equal
    ADD = mybir.AluOpType.add

    TC = sched.total
    nc = bacc.Bacc(
        "TRN2",
        target_bir_lowering=False,
        debug=False,
        num_devices=cfg.CORES,
        num_swdge_queues=4,
    )

    # ---- I/O ----
    xT_d = nc.dram_tensor("xT", [cfg.IN, cfg.NPAD], f32, kind="ExternalInput")
    W_d = nc.dram_tensor("Wm", [cfg.IN, cfg.HID], f32, kind="ExternalInput")
    brep_d = nc.dram_tensor("brep", [128, cfg.HID], f32, kind="ExternalInput")
    iota_d = nc.dram_tensor("iota", [128, 128], f32, kind="ExternalInput")
    normc_d = nc.dram_tensor("normc", [128, cfg.BPC], f32, kind="ExternalInput")
    norm2c_d = nc.dram_tensor("norm2c", [128, cfg.BPC], f32, kind="ExternalInput")
    idx_d = nc.dram_tensor("idx_all", [128, TC * 8], i16, kind="ExternalInput")
    rr_d = nc.dram_tensor("rr_all", [128, TC], f32, kind="ExternalInput")
    out_d = nc.dram_tensor("out", [cfg.NPAD, cfg.HID], f32, kind="ExternalOutput")

    # internal DRAM: AG input (own xs rows) + AG output (full table), per layer
    xs_in = [
        nc.dram_tensor(f"xs_in{l}", [cfg.NPAD, cfg.HID], f32)
        for l in range(cfg.LAYERS)
    ]
    # Shared-output AG is only supported for >4-core groups
    _aspace = "Shared" if cfg.CORES > 4 else "Local"
    NTAB = cfg.CORES * cfg.NPAD  # padded table rows
    xs_full = [
        nc.dram_tensor(
            f"xs_full{l}", [NTAB, cfg.HID], f32, addr_space=_aspace
        )
        for l in range(cfg.LAYERS)
    ]

    rg = [list(range(cfg.CORES))]

    with tile.TileContext(nc) as tc:
        with (
            tc.tile_pool(name="const", bufs=1) as constp,
            tc.tile_pool(name="gbuf", bufs=cfg.GBUFS) as gpool,
            tc.tile_pool(name="onehot", bufs=2) as opool,
            tc.tile_pool(name="xsg", bufs=2) as xsgp,
            tc.tile_pool(name="psum", bufs=8, space="PSUM") as psp,
        ):
            # ---- persistent SBUF ----
            W_s = constp.tile([cfg.IN, cfg.HID], f32, tag="W")
            brep_s = constp.tile([128, cfg.HID], f32, tag="brep")
            iota_s = constp.tile([128, 128], f32, tag="iota")
            normc_s = constp.tile([128, cfg.BPC], f32, tag="normc")
            norm2c_s = constp.tile([128, cfg.BPC], f32, tag="norm2c")
            idx_s = constp.tile([128, TC * 8], i16, tag="idx")
            rr_s = constp.tile([128, TC], f32, tag="rr")
            xs_ping = constp.tile([128, cfg.BPC, cfg.HID], f32, tag="xsA")
            xs_pong = constp.tile([128, cfg.BPC, cfg.HID], f32, tag="xsB")

            nc.sync.dma_start(W_s[:], W_d[:, :])
            nc.sync.dma_start(brep_s[:], brep_d[:, :])
            nc.sync.dma_start(iota_s[:], iota_d[:, :])
            nc.sync.dma_start(normc_s[:], normc_d[:, :])
            nc.sync.dma_start(norm2c_s[:], norm2c_d[:, :])
            nc.sync.dma_start(idx_s[:], idx_d[:, :])
            nc.sync.dma_start(rr_s[:], rr_d[:, :])

            def store_group_to_dram(dram, g, src_tile, nhid):
                """src_tile [128, nb, nhid] -> dram rows [128*b0, 128*b1)."""
                blocks = cfg.group_blocks(g)
                b0, b1 = blocks[0], blocks[-1] + 1
                dst = dram[128 * b0 : 128 * b1, :].rearrange(
                    "(b p) h -> p b h", p=128
                )
                nc.sync.dma_start(dst, src_tile[:, : b1 - b0, :])

            # ---- prologue: h0 = x @ W + b ; xs0 = norm * h0 ----
            with tc.tile_pool(name="xtp", bufs=2) as xtp:
                xT_s = constp.tile([cfg.IN, cfg.NPAD], f32, tag="xT")
                nc.sync.dma_start(xT_s[:], xT_d[:, :])
                for g in range(cfg.NGROUPS):
                    blocks = cfg.group_blocks(g)
                    xs_g = xsgp.tile([128, len(blocks), cfg.HID], f32, tag="xsg")
                    for j, b in enumerate(blocks):
                        ps = psp.tile([128, cfg.HID], f32, tag="ps")
                        nc.tensor.matmul(
                            ps[:],
                            xT_s[:, 128 * b : 128 * (b + 1)],
                            W_s[:],
                            start=True,
                            stop=True,
                        )
                        tmp = xtp.tile([128, cfg.HID], f32, tag="t0")
                        nc.vector.tensor_tensor(tmp[:], ps[:], brep_s[:], ADD)
                        nc.vector.tensor_scalar_mul(
                            xs_g[:, j, :], tmp[:], normc_s[:, b : b + 1]
                        )
                        nc.vector.tensor_copy(xs_ping[:, b, :], xs_g[:, j, :])
                    store_group_to_dram(xs_in[0], g, xs_g, cfg.HID)

            if not no_ag:
                nc.gpsimd.collective_compute(
                    "AllGather",
                    mybir.AluOpType.bypass,
                    replica_groups=rg,
                    ins=[xs_in[0][:, :]],
                    outs=[xs_full[0][:, :]],
                )

            # ---- layers ----
            xs_cur, xs_nxt = xs_ping, xs_pong
            for _rep in range(cfg.REPEAT):
              for l in range(cfg.LAYERS):
                last = l == cfg.LAYERS - 1
                table = xs_full[l]
                for g in range(cfg.NGROUPS):
                    blocks = cfg.group_blocks(g)
                    calls = sched.group_call_slots[g]
                    gt = {}
                    for h in (0, 1):
                        s0, s1 = calls[h]
                        nch = s1 - s0
                        if nch == 0:
                            continue
                        G = gpool.tile([128, nch, cfg.HID], f32, tag=f"G{h}")
                        src = (
                            table[: cfg.HALF, :]
                            if h == 0
                            else table[cfg.HALF :, :]
                        )
                        # split into sub-calls: the gather ucode's descriptor
                        # ring chokes on >~1.3k idxs per instruction
                        oh = opool.tile([128, nch, 128], f32, tag=f"oh{h}")
                        for ci, o0 in enumerate(
                            [] if "gather" in ablate else range(0, nch, cfg.MAXC)
                        ):
                            o1 = min(o0 + cfg.MAXC, nch)
                            n = o1 - o0
                            nc.gpsimd.dma_gather(
                                G[:, o0:o1, :],
                                src,
                                idx_s[:, 8 * (s0 + o0) : 8 * (s0 + o1)],
                                n * 128,
                                n * 128,
                                cfg.HID,
                                single_packet=False,
                                queue_num=(ci + 2 * h) % 4,
                            )
                        _ohstep = cfg.MAXC if cfg.OHCHUNK else nch
                        for o0 in range(0, nch, _ohstep):
                            o1 = min(o0 + _ohstep, nch)
                            n = o1 - o0
                            rr_b = (
                                rr_s[:, s0 + o0 : s0 + o1]
                                .unsqueeze(2)
                                .broadcast_to([128, n, 128])
                            )
                            io_b = (
                                iota_s[:, :]
                                .unsqueeze(1)
                                .broadcast_to([128, n, 128])
                            )
                            if "oh" not in ablate:
                                nc.vector.tensor_tensor(
                                    oh[:, o0:o1, :], io_b, rr_b, EQ
                                )
                        gt[h] = (G, oh, s0)

                    xs_g = xsgp.tile([128, len(blocks), cfg.HID], f32, tag="xsg")
                    for j, b in enumerate(blocks):
                        ps = psp.tile([128, cfg.HID], f32, tag="ps")
                        # all slots of this block, halves concatenated
                        mm = []
                        for h in (0, 1):
                            bs0, bs1 = sched.block_slot_ranges[b][h]
                            if bs1 > bs0:
                                G, oh, s0 = gt[h]
                                for s in range(bs0, bs1):
                                    mm.append((G, oh, s - s0))
                        assert mm, f"block {b} has no slots"
                        if "mm" in ablate:
                            nc.vector.memset(ps[:], 0.0)
                            mm = []
                        for k, (G, oh, sl) in enumerate(mm):
                            nc.tensor.matmul(
                                ps[:],
                                oh[:, sl, :],
                                G[:, sl, :],
                                start=(k == 0),
                                stop=(k == len(mm) - 1),
                            )
                        # out = (agg + xs_cur[b]) * (norm2 or norm)
                        nsrc = normc_s if last else norm2c_s
                        tmp = xsgp.tile([128, cfg.HID], f32, tag="t1")
                        nc.vector.tensor_tensor(
                            tmp[:], ps[:], xs_cur[:, b, :], ADD
                        )
                        nc.vector.tensor_scalar_mul(
                            xs_g[:, j, :], tmp[:], nsrc[:, b : b + 1]
                        )
                        if not last:
                            nc.vector.tensor_copy(xs_nxt[:, b, :], xs_g[:, j, :])
                    if last:
                        store_group_to_dram(out_d, g, xs_g, cfg.HID)
                    else:
                        store_group_to_dram(xs_in[l + 1], g, xs_g, cfg.HID)

                if not last:
                    if not no_ag:
                        nc.gpsimd.collective_compute(
                            "AllGather",
                            mybir.AluOpType.bypass,
                            replica_groups=rg,
                            ins=[xs_in[l + 1][:, :]],
                            outs=[xs_full[l + 1][:, :]],
                        )
                    xs_cur, xs_nxt = xs_nxt, xs_cur

    nc.compile()
    return nc


def bench_bass(nc, in_maps, n_cores, iters=20, warmup=2):
    """Repeat-execution device benchmark (no NTFF tracing in this container).

    Mirrors bass2jax.run_bass_via_pjrt's multi-core path, minus output-buffer
    donation so the compiled executable can be re-invoked. Returns
    (results_list, per_iter_seconds).
    """
    import time

    import jax
    from jax.experimental.shard_map import shard_map
    from jax.sharding import Mesh, NamedSharding, PartitionSpec

    from concourse import bass2jax, mybir

    bass2jax.install_neuronx_cc_hook()

    partition_name = (
        nc.partition_id_tensor.name if nc.partition_id_tensor else None
    )
    in_names, out_names, out_avals, zero_outs = [], [], [], []
    for alloc in nc.m.functions[0].allocations:
        if not isinstance(alloc, mybir.MemoryLocationSet):
            continue
        name = alloc.memorylocations[0].name
        if alloc.kind == "ExternalInput":
            if name != partition_name:
                in_names.append(name)
        elif alloc.kind == "ExternalOutput":
            out_names.append(name)
            shape = tuple(alloc.tensor_shape)
            dtype = mybir.dt.np(alloc.dtype)
            out_avals.append(jax.core.ShapedArray(shape, dtype))
            zero_outs.append(np.zeros(shape, dtype))
    n_params = len(in_names)
    all_names = in_names + out_names
    if partition_name is not None:
        all_names = all_names + [partition_name]

    def _body(*args):
        operands = list(args)
        if partition_name is not None:
            operands.append(bass2jax.partition_id_tensor())
        outs = bass2jax._bass_exec_p.bind(
            *operands,
            out_avals=tuple(out_avals),
            in_names=tuple(all_names),
            out_names=tuple(out_names),
            lowering_input_output_aliases=(),
            sim_require_finite=True,
            sim_require_nnan=True,
            nc=nc,
        )
        return tuple(outs)

    devices = jax.devices()[:n_cores]
    mesh = Mesh(np.asarray(devices), ("core",))
    spec = PartitionSpec("core")
    nin = n_params + len(zero_outs)
    sharded = jax.jit(
        shard_map(
            _body,
            mesh=mesh,
            in_specs=(spec,) * nin,
            out_specs=(spec,) * len(out_names),
            check_rep=False,
        ),
        keep_unused=True,
    )
    sh = NamedSharding(mesh, spec)
    args = [
        jax.device_put(
            np.concatenate([np.asarray(m[name]) for m in in_maps], axis=0), sh
        )
        for name in in_names
    ] + [
        jax.device_put(
            np.zeros((n_cores * z.shape[0], *z.shape[1:]), z.dtype), sh
        )
        for z in zero_outs
    ]

    out_arrs = None
    for _ in range(warmup):
        out_arrs = sharded(*args)
        jax.block_until_ready(out_arrs)
    t0 = time.perf_counter()
    for _ in range(iters):
        out_arrs = sharded(*args)
    jax.block_until_ready(out_arrs)
    t1 = time.perf_counter()

    results = [
        {
            name: np.asarray(out_arrs[i]).reshape(n_cores, *out_avals[i].shape)[c]
            for i, name in enumerate(out_names)
        }
        for c in range(n_cores)
    ]
    return results, (t1 - t0) / iters


def kernel(x, edge_index, W, b, cfg: Cfg | None = None, trace: bool = False):
    """Full-input entry point. Returns [N, HID] float32 (+ results if trace)."""
    cfg = cfg or Cfg()
    sched, in_maps = host_inputs(x, edge_index, W, b, cfg)
    nc = build_bass(cfg, sched)

    from concourse import bass_utils

    res = bass_utils.run_bass_kernel_spmd(
        nc,
        in_maps,
        core_ids=list(range(cfg.CORES)),
        trace=False,
    )
    out = np.concatenate(
        [r["out"][: cfg.NPC] for r in res.results], axis=0
    )
    if trace:
        return out, res
    return out


if __name__ == "__main__":
    pass
